# revision 31
# baseline (speedup 1.0000x reference)
"""W8A16 column-parallel linear for TRN2, 8 NeuronCores.

Computes y = x @ (qweight * w_scales).T + bias with
  x        [8, 1, 8192]  fp16
  qweight  [28672, 8192] int8 (per-row symmetric quant)
  w_scales [28672, 1]    fp16
  bias     [28672]       fp16
  y        [8, 1, 28672] fp16

Sharding: column-parallel - each of the 8 cores owns 3584 output rows
(qweight/w_scales/bias shard), x replicated. No collectives; outputs are
concatenated on the host.

Per-core kernel, span-major streaming: the 3584 output columns are cut
into 7 spans of 512. The int8 weight shard streams span-by-span from HBM
(host pre-arranges each DMA group as a contiguous [128, u*512] slab so
descriptors are u*512 B per partition), is converted int8->fp16 on-chip
(k-split between VectorE 2x-port mode and ScalarE), and accumulates into
that span's PSUM bank with fp16 matmuls (stationary x^T tile, moving
weight tile). Each span is split into 2x256-col halves on different PE
column groups so even a HAM-cold PE outruns the DMA pace. When a span's
64 k-tiles finish, its scale-multiply (out=(sum x*q + b/s)*s) and output
DMA run *under* the next span's weight stream - unlike the k-major
baseline whose 3 serial full-width scale-muls + output DMAs formed an
~18 us tail after the last weight byte. The last span tapers its group
sizes so the drain after the final weight byte is short.

Measured on TRN2 (8 cores, neuron-profile total_time, median of 3):
~102.3us vs ~106us for the k-major baseline; rel err 3.4e-4; weight
stream gap-free at 99.4% of the 358 GB/s HBM line rate. Floor
decomposition: ~8.7us runtime startup + ~89-90us PE-paced stream +
~3us drain/out. The PE is the binding stage at ~2.58 weight-cols/ns:
the moving-operand path delivers ~1.08 cols/cycle with the
two-column-group pairing (drain/fill + LDW overlap is a pairwise-only
effect worth ~75ns/pair).

Falsified on HW, do not retry blindly:
- mid-stream scale-muls or output DMAs (any op whose semaphore waits
  on the lagging PE stalls the strict-FIFO DVE queue / HWDGE
  sequencer, idles the PE >3.4us, the HAM clock gate halves its
  clock, and the pipeline death-spirals: +15-25us even when deferred
  by 2 spans; hence ALL muls/outs issue after the full stream)
- 3-way column-group splits at BOTH 171 cols (1.77 cols/ns, run5)
  and 512 cols / 1536-wide spans (110.1us vs 102.3; the pairwise
  overlap does not chain to a third stream)
- 1024-col moving tiles (s3d3_mm_num_elements ISA reject)
- --enable-ldw-opt=true (hard device hang once, slower when it ran)
- bf16 matmul path (identical 102.5us - no 2-elem/cycle moving path;
  fp16 kept for 5x better rel err)
- one merged/padded weight param (121us: slice-of-param APs degrade
  the DMA descriptor program; only whole-param rearrange views
  stream at line rate)

Untested but promising for a future session: standalone
nc.tensor.ldweights + hand-built weights-less InstMatmult
(ins=[ifmap] only - the non-self-loading pattern referenced in the
ldweights docstring, valid for non-fp32) to collapse the per-ktile
LDW pair and chain both halves' matmuls with no LDW between them
(~-2us); and the ~8.7us runtime startup, which sits below the
NEFF/NRT layer reachable from bass.
"""

import numpy as np

import concourse.bacc as bacc
import concourse.mybir as mybir
import concourse.tile as tile
from concourse.bass_utils import run_bass_kernel_spmd

B, S, K, N = 8, 1, 8192, 28672
M = B * S                 # 8 rows in the GEMM
NCORES = 8
NS = N // NCORES          # 3584 output rows per core
KT = K // 128             # 64 k-tiles
# spans 0-2 are 1024 cols (two 512-col halves on PE column groups 0/1 -
# alternating the two moving streams overlaps them ~50% on the PE);
# span 3 is 512 cols (two 256-col halves). Per-span k-group sizes: the
# last span tapers so the pipeline drains fast after the final byte.
SPANS = [(0, 1024, [8] * 8),
         (1024, 1024, [8] * 8),
         (2048, 1024, [8] * 8),
         (3072, 512, [16, 16, 16, 8, 4, 2, 1, 1])]
NSPAN = len(SPANS)
for _, _, gs in SPANS:
    assert sum(gs) == KT

# k-split of each group's int8->fp16 conversion: VectorE (2 elem/cyc)
# takes dve_u() k-tiles, ScalarE (1 elem/cyc @1.2GHz) the rest.
DVE_U = {(8, 1024): 5, (16, 512): 10, (8, 512): 5, (4, 512): 3,
         (2, 512): 2, (1, 512): 1}


def dve_u(u, w):
    return DVE_U[(u, w)]

_CACHE = {}


def _u_classes():
    """((u, width) -> number of groups of that class in the kernel)."""
    cnt = {}
    for _, w, gs in SPANS:
        for u in gs:
            cnt[(u, w)] = cnt.get((u, w), 0) + 1
    return cnt


def _build():
    # NOTE: do NOT flip --enable-ldw-opt=true: it hard-hangs the device
    # (NRT_EXEC_UNIT_UNRECOVERABLE) with this instruction pattern.
    nc = bacc.Bacc()
    xp = nc.declare_dram_parameter("x", [128, KT * M + M], mybir.dt.float16,
                                   isOutput=False)
    ucnt = _u_classes()
    qps = {
        (u, w): nc.declare_dram_parameter(f"q{u}w{w}", [n, 128, u * w],
                                          mybir.dt.int8, isOutput=False)
        for (u, w), n in sorted(ucnt.items())
    }
    sp_ = nc.declare_dram_parameter("s", [M, NS], mybir.dt.float16, isOutput=False)
    bp = nc.declare_dram_parameter("b", [1, NS], mybir.dt.float16, isOutput=False)
    op = nc.declare_dram_parameter("out", [M, NS], mybir.dt.float16, isOutput=True)
    # whole-param rearranges: keep the (u n) free dim contiguous per
    # partition so each group DMA is 128 descriptors of u*512 bytes
    qv = {k: qps[k].rearrange("g p (u n) -> g p u n", u=k[0]) for k in qps}

    with tile.TileContext(nc) as tc:
        with (
            tc.tile_pool(name="const", bufs=1) as constp,
            tc.tile_pool(name="wq", bufs=6) as wqp,
            tc.tile_pool(name="wf", bufs=4) as wfp,
            tc.tile_pool(name="psum", bufs=1, space="PSUM") as psp,
        ):
            xsb = constp.tile([128, KT * M + M], mybir.dt.float16, tag="xsb")
            sb = constp.tile([40, NS], mybir.dt.float16, tag="sb")
            b1 = constp.tile([1, NS], mybir.dt.float16, tag="b1")
            osb = constp.tile([40, NS], mybir.dt.float16, tag="osb")
            # ones row for the bias-opening matmuls lives in xsb's last
            # M columns (host packs 1.0 at partition 0 there)
            ones = xsb[0:1, KT * M:KT * M + M]

            psum = psp.tile([128, NS], mybir.dt.float32, tag="psum")

            # the weight stream is the binding resource: its first group
            # leads the HWDGE queue, constants ride behind it
            qidx = {k: 0 for k in ucnt}
            k0 = (SPANS[0][2][0], SPANS[0][1])
            wq0 = wqp.tile([128, k0[0], k0[1]], mybir.dt.int8, tag="wq")
            nc.sync.dma_start(wq0[:], qv[k0][0])
            qidx[k0] += 1
            nc.sync.dma_start(xsb[:], xp[:])
            nc.sync.dma_start(b1[:], bp[:])
            for j in range(2):
                nc.sync.dma_start(sb[32 * j:32 * j + M, :], sp_[:])

            def halves(sp):
                c, w, _ = SPANS[sp]
                return [(0, c, c + w // 2), (32, c + w // 2, c + w)]

            def mm_nw(out, lhsT, rhs, tile_position, start, stop):
                """Non-self-loading matmul: uses the stationary operand
                loaded by a preceding nc.tensor.ldweights (valid for
                non-fp32 dtypes per bass's ldweights docstring). Keeps
                the two halves' matmuls adjacent on the PE queue with no
                LDWEIGHTS between them."""
                eng = nc.tensor
                ifmap_ap = eng.lower_ap(rhs.opt({0}), opt=False)
                weights_ap = eng.lower_ap(lhsT.opt({0}), opt=False,
                                          for_matmul_weights=True)
                out_ap = eng.lower_ap(out)
                return eng.add_instruction(mybir.InstMatmult(
                    name=eng.bass.get_next_instruction_name(),
                    replication_resolution=0,
                    replication_shift_amnt=0,
                    replication_num_rows=0,
                    start_tensor_calc=start,
                    stop_tensor_calc=stop,
                    ins=[ifmap_ap, weights_ap],
                    outs=[out_ap],
                    perf_mode=None,
                    is_transpose=None,
                    ifmap_quant_offset=None,
                    weights_quant_offset=None,
                    bass_skip_group_check=False,
                    tile_position=tile_position,
                    tile_size=(128, 32),
                    ldweights=False,
                ))

            def span_muls(sp):
                for p, c0, c1 in halves(sp):
                    nc.vector.tensor_mul(
                        osb[p:p + M, c0:c1],
                        psum[p:p + M, c0:c1],
                        sb[p:p + M, c0:c1])

            for sp in range(NSPAN):
                c, w, groups = SPANS[sp]
                # bias rows open the accumulation: psum = ones^T @ (b/s)
                for p, c0, c1 in halves(sp):
                    nc.tensor.matmul(psum[p:p + M, c0:c1], ones,
                                     b1[:, c0:c1], start=True, stop=False)

                kt0 = 0
                for g, u in enumerate(groups):
                    if sp == 0 and g == 0:
                        wq = wq0
                    else:
                        wq = wqp.tile([128, u, w], mybir.dt.int8, tag="wq")
                        nc.sync.dma_start(wq[:], qv[(u, w)][qidx[(u, w)]])
                        qidx[(u, w)] += 1
                    wf = wfp.tile([128, u, w], mybir.dt.float16, tag="wf")
                    u1 = dve_u(u, w)
                    nc.vector.tensor_copy(wf[:, 0:u1, :], wq[:, 0:u1, :])
                    if u1 < u:
                        nc.scalar.activation(
                            wf[:, u1:u, :], wq[:, u1:u, :],
                            mybir.ActivationFunctionType.Copy,
                        )
                    for ui in range(u):
                        kt = kt0 + ui
                        last = kt == KT - 1
                        xt = xsb[:, kt * M:(kt + 1) * M]
                        # load the x stationary into both column groups,
                        # then issue both halves' matmuls back-to-back
                        # with no LDWEIGHTS between them
                        for p, _, _ in halves(sp):
                            nc.tensor.ldweights(xt, tile_position=(0, p))
                        for p, c0, c1 in halves(sp):
                            mm_nw(psum[p:p + M, c0:c1], xt,
                                  wf[:, ui, c0 - c:c1 - c],
                                  tile_position=(0, p),
                                  start=False, stop=last)
                    kt0 += u

            # ALL scale-muls run after the stream: the PE is the
            # slowest pipeline stage (~2.6 weight-cols/ns vs the DMA's
            # ~2.8), so a mid-stream mul on the strict-FIFO DVE queue
            # waits on the lagging PE and starves the conversions the
            # PE is waiting for (cold-clock death spiral, measured
            # +15us). At the end they stall nothing.
            for sp in range(NSPAN):
                span_muls(sp)

            # output DMAs are batched AFTER the whole stream: a dma_start
            # makes the issuing sequencer wait on its source semaphore, so
            # a per-span out DMA would stall HWDGE descriptor generation
            # for every weight group queued behind it (measured: ~25 us).
            # Here they stall nothing, and all but the last span's fire
            # immediately.
            for sp in range(NSPAN):
                for p, c0, c1 in halves(sp):
                    nc.sync.dma_start(op[:, c0:c1], osb[p:p + M, c0:c1])

    nc.compile()
    return nc


def _get_nc():
    if "nc" not in _CACHE:
        _CACHE["nc"] = _build()
    return _CACHE["nc"]


def _prep_inputs(x, qweight, w_scales, bias):
    x2 = np.asarray(x, dtype=np.float16).reshape(M, K)
    # xsb[p, kt*M + m] = x[m, kt*128 + p]; last M cols: ones row at p=0
    xsb = np.zeros((128, KT * M + M), dtype=np.float16)
    xsb[:, :KT * M] = x2.T.reshape(KT, 128, M).transpose(1, 0, 2).reshape(128, KT * M)
    xsb[0, KT * M:] = 1.0
    qweight = np.asarray(qweight)
    w_scales = np.asarray(w_scales, dtype=np.float16).reshape(N)
    bias = np.asarray(bias, dtype=np.float16).reshape(N)
    ucnt = _u_classes()
    in_maps = []
    for c in range(NCORES):
        sl = slice(c * NS, (c + 1) * NS)
        # A[kt, p, n] = q[n, kt*128+p]
        A = np.ascontiguousarray(qweight[sl, :].T).reshape(KT, 128, NS)
        qarr = {k: np.empty((n, 128, k[0] * k[1]), dtype=np.int8)
                for k, n in ucnt.items()}
        qidx = {k: 0 for k in ucnt}
        for c0, w, groups in SPANS:
            kt0 = 0
            for u in groups:
                blk = A[kt0:kt0 + u, :, c0:c0 + w].transpose(1, 0, 2)
                qarr[(u, w)][qidx[(u, w)]] = blk.reshape(128, u * w)
                qidx[(u, w)] += 1
                kt0 += u
        srep = np.broadcast_to(w_scales[sl], (M, NS)).astype(np.float16)
        # bias enters the PSUM accumulation before the scale multiply:
        # out = (sum x*q + b/s) * s
        bos = (bias[sl].astype(np.float32)
               / w_scales[sl].astype(np.float32)).astype(np.float16)
        im = {"x": xsb, "s": np.ascontiguousarray(srep),
              "b": np.ascontiguousarray(bos.reshape(1, NS))}
        for (u, w) in ucnt:
            im[f"q{u}w{w}"] = qarr[(u, w)]
        in_maps.append(im)
    return in_maps


def _run(x, qweight, w_scales, bias, trace=False):
    nc = _get_nc()
    in_maps = _prep_inputs(x, qweight, w_scales, bias)
    res = run_bass_kernel_spmd(
        nc, in_maps, core_ids=list(range(NCORES)), trace=trace
    )
    y = np.concatenate(
        [np.asarray(res.results[c]["out"]) for c in range(NCORES)], axis=1
    )
    return y.reshape(B, S, N).astype(np.float16), res


def kernel(x, qweight, w_scales, bias):
    y, _ = _run(x, qweight, w_scales, bias, trace=False)
    return y


def kernel_traced(x, qweight, w_scales, bias):
    """Like kernel() but also returns the BassKernelResults (exec_time_ns)."""
    return _run(x, qweight, w_scales, bias, trace=True)


# revision 33
# speedup vs baseline: 1.0634x; 1.0634x over previous
"""W8A16 column-parallel linear for TRN2, 8 NeuronCores.

Computes y = x @ (qweight * w_scales).T + bias with
  x        [8, 1, 8192]  fp16
  qweight  [28672, 8192] int8 (per-row symmetric quant)
  w_scales [28672, 1]    fp16
  bias     [28672]       fp16
  y        [8, 1, 28672] fp16

Sharding: column-parallel - each of the 8 cores owns 3584 output rows
(qweight/w_scales/bias shard), x replicated. No collectives; outputs are
concatenated on the host.

Per-core kernel, span-major streaming: the 3584 output columns are cut
into 7 spans of 512. The int8 weight shard streams span-by-span from HBM
(host pre-arranges each DMA group as a contiguous [128, u*512] slab so
descriptors are u*512 B per partition), is converted int8->fp16 on-chip
(k-split between VectorE 2x-port mode and ScalarE), and accumulates into
that span's PSUM bank with fp16 matmuls (stationary x^T tile, moving
weight tile). Each span is split into 2x256-col halves on different PE
column groups so even a HAM-cold PE outruns the DMA pace. When a span's
64 k-tiles finish, its scale-multiply (out=(sum x*q + b/s)*s) and output
DMA run *under* the next span's weight stream - unlike the k-major
baseline whose 3 serial full-width scale-muls + output DMAs formed an
~18 us tail after the last weight byte. The last span tapers its group
sizes so the drain after the final weight byte is short.

Measured on TRN2 (8 cores, neuron-profile total_time, median of 3):
~102.3us vs ~106us for the k-major baseline; rel err 3.4e-4; weight
stream gap-free at 99.4% of the 358 GB/s HBM line rate. Floor
decomposition: ~8.7us runtime startup + ~89-90us PE-paced stream +
~3us drain/out. The PE is the binding stage at ~2.58 weight-cols/ns
(~1.08 cols/cycle with the two-column-group pairing; the drain/fill +
LDW overlap is a pairwise-only effect worth ~75ns/pair).

Falsified on HW, do not retry blindly:
- mid-stream scale-muls or output DMAs (any op whose semaphore waits
  on the lagging PE stalls the strict-FIFO DVE queue / HWDGE
  sequencer, idles the PE >3.4us, the HAM clock gate halves its
  clock, and the pipeline death-spirals: +15-25us even when deferred
  by 2 spans; hence ALL muls/outs issue after the full stream)
- 3-way column-group splits at BOTH 171 cols (1.77 cols/ns) and
  512 cols / 1536-wide spans (110.1us; the pairwise overlap does not
  chain to a third stream)
- 1024-col moving tiles (s3d3_mm_num_elements ISA reject)
- --enable-ldw-opt=true (hard device hang once, slower when it ran)
- bf16 matmul path (identical 102.5us - no 2-elem/cycle moving path;
  fp16 kept for 5x better rel err)
- one merged/padded weight param (121us: slice-of-param APs degrade
  the DMA descriptor program; only whole-param rearrange views
  stream at line rate)
- standalone nc.tensor.ldweights + InstMatmult(ldweights=False)
  (the non-self-loading pattern builds and is CORRECT on HW, but
  measures 128.2us - the explicit LDW pairs serialize worse than the
  fused LDW+MM emission; keep fused matmul())

Remaining slack for a future session: the ~8.7us runtime startup
(below the NEFF/NRT layer reachable from bass) and the PE
moving-stream ceiling (needs xbus_sel control, not exposed in bass).
"""

import numpy as np

import concourse.bacc as bacc
import concourse.mybir as mybir
import concourse.tile as tile
from concourse.bass_utils import run_bass_kernel_spmd

B, S, K, N = 8, 1, 8192, 28672
M = B * S                 # 8 rows in the GEMM
NCORES = 8
NS = N // NCORES          # 3584 output rows per core
KT = K // 128             # 64 k-tiles
# spans 0-2 are 1024 cols (two 512-col halves on PE column groups 0/1 -
# alternating the two moving streams overlaps them ~50% on the PE);
# span 3 is 512 cols (two 256-col halves). Per-span k-group sizes: the
# last span tapers so the pipeline drains fast after the final byte.
SPANS = [(0, 1024, [8] * 8),
         (1024, 1024, [8] * 8),
         (2048, 1024, [8] * 8),
         (3072, 512, [16, 16, 16, 8, 4, 2, 1, 1])]
NSPAN = len(SPANS)
for _, _, gs in SPANS:
    assert sum(gs) == KT

# k-split of each group's int8->fp16 conversion: VectorE (2 elem/cyc)
# takes dve_u() k-tiles, ScalarE (1 elem/cyc @1.2GHz) the rest.
DVE_U = {(8, 1024): 5, (16, 512): 10, (8, 512): 5, (4, 512): 3,
         (2, 512): 2, (1, 512): 1}


def dve_u(u, w):
    return DVE_U[(u, w)]

_CACHE = {}


def _u_classes():
    """((u, width) -> number of groups of that class in the kernel)."""
    cnt = {}
    for _, w, gs in SPANS:
        for u in gs:
            cnt[(u, w)] = cnt.get((u, w), 0) + 1
    return cnt


def _build():
    # NOTE: do NOT flip --enable-ldw-opt=true: it hard-hangs the device
    # (NRT_EXEC_UNIT_UNRECOVERABLE) with this instruction pattern.
    nc = bacc.Bacc()
    xp = nc.declare_dram_parameter("x", [128, KT * M + M], mybir.dt.float16,
                                   isOutput=False)
    ucnt = _u_classes()
    qps = {
        (u, w): nc.declare_dram_parameter(f"q{u}w{w}", [n, 128, u * w],
                                          mybir.dt.int8, isOutput=False)
        for (u, w), n in sorted(ucnt.items())
    }
    sp_ = nc.declare_dram_parameter("s", [M, NS], mybir.dt.float16, isOutput=False)
    bp = nc.declare_dram_parameter("b", [1, NS], mybir.dt.float16, isOutput=False)
    op = nc.declare_dram_parameter("out", [M, NS], mybir.dt.float16, isOutput=True)
    # whole-param rearranges: keep the (u n) free dim contiguous per
    # partition so each group DMA is 128 descriptors of u*512 bytes
    qv = {k: qps[k].rearrange("g p (u n) -> g p u n", u=k[0]) for k in qps}

    with tile.TileContext(nc) as tc:
        with (
            tc.tile_pool(name="const", bufs=1) as constp,
            tc.tile_pool(name="wq", bufs=7) as wqp,
            tc.tile_pool(name="wf", bufs=6) as wfp,
            tc.tile_pool(name="psum", bufs=1, space="PSUM") as psp,
        ):
            xsb = constp.tile([128, KT * M + M], mybir.dt.float16, tag="xsb")
            sb = constp.tile([40, NS], mybir.dt.float16, tag="sb")
            b1 = constp.tile([1, NS], mybir.dt.float16, tag="b1")
            osb = constp.tile([40, NS], mybir.dt.float16, tag="osb")
            # ones row for the bias-opening matmuls lives in xsb's last
            # M columns (host packs 1.0 at partition 0 there)
            ones = xsb[0:1, KT * M:KT * M + M]

            psum = psp.tile([128, NS], mybir.dt.float32, tag="psum")

            # the weight stream is the binding resource: its first group
            # leads the HWDGE queue, constants ride behind it
            qidx = {k: 0 for k in ucnt}
            k0 = (SPANS[0][2][0], SPANS[0][1])
            wq0 = wqp.tile([128, k0[0], k0[1]], mybir.dt.int8, tag="wq")
            nc.sync.dma_start(wq0[:], qv[k0][0])
            qidx[k0] += 1
            nc.sync.dma_start(xsb[:], xp[:])
            nc.sync.dma_start(b1[:], bp[:])
            for j in range(2):
                nc.sync.dma_start(sb[32 * j:32 * j + M, :], sp_[:])

            def halves(sp):
                c, w, _ = SPANS[sp]
                return [(0, c, c + w // 2), (32, c + w // 2, c + w)]

            def span_muls(sp):
                for p, c0, c1 in halves(sp):
                    nc.vector.tensor_mul(
                        osb[p:p + M, c0:c1],
                        psum[p:p + M, c0:c1],
                        sb[p:p + M, c0:c1])

            for sp in range(NSPAN):
                c, w, groups = SPANS[sp]
                # bias rows open the accumulation: psum = ones^T @ (b/s)
                for p, c0, c1 in halves(sp):
                    nc.tensor.matmul(psum[p:p + M, c0:c1], ones,
                                     b1[:, c0:c1], start=True, stop=False)

                kt0 = 0
                for g, u in enumerate(groups):
                    if sp == 0 and g == 0:
                        wq = wq0
                    else:
                        wq = wqp.tile([128, u, w], mybir.dt.int8, tag="wq")
                        nc.sync.dma_start(wq[:], qv[(u, w)][qidx[(u, w)]])
                        qidx[(u, w)] += 1
                    wf = wfp.tile([128, u, w], mybir.dt.float16, tag="wf")
                    u1 = dve_u(u, w)
                    nc.vector.tensor_copy(wf[:, 0:u1, :], wq[:, 0:u1, :])
                    if u1 < u:
                        nc.scalar.activation(
                            wf[:, u1:u, :], wq[:, u1:u, :],
                            mybir.ActivationFunctionType.Copy,
                        )
                    for ui in range(u):
                        kt = kt0 + ui
                        last = kt == KT - 1
                        xt = xsb[:, kt * M:(kt + 1) * M]
                        # alternate the two halves' matmuls so the PE
                        # overlaps the two moving streams
                        for p, c0, c1 in halves(sp):
                            nc.tensor.matmul(
                                psum[p:p + M, c0:c1], xt,
                                wf[:, ui, c0 - c:c1 - c],
                                start=False, stop=last)
                    kt0 += u

            # ALL scale-muls run after the stream: the PE is the
            # slowest pipeline stage (~2.6 weight-cols/ns vs the DMA's
            # ~2.8), so a mid-stream mul on the strict-FIFO DVE queue
            # waits on the lagging PE and starves the conversions the
            # PE is waiting for (cold-clock death spiral, measured
            # +15us). At the end they stall nothing.
            for sp in range(NSPAN):
                span_muls(sp)

            # output DMAs are batched AFTER the whole stream: a dma_start
            # makes the issuing sequencer wait on its source semaphore, so
            # a per-span out DMA would stall HWDGE descriptor generation
            # for every weight group queued behind it (measured: ~25 us).
            # Here they stall nothing, and all but the last span's fire
            # immediately.
            for sp in range(NSPAN):
                for p, c0, c1 in halves(sp):
                    nc.sync.dma_start(op[:, c0:c1], osb[p:p + M, c0:c1])

    nc.compile()
    return nc


def _get_nc():
    if "nc" not in _CACHE:
        _CACHE["nc"] = _build()
    return _CACHE["nc"]


def _prep_inputs(x, qweight, w_scales, bias):
    x2 = np.asarray(x, dtype=np.float16).reshape(M, K)
    # xsb[p, kt*M + m] = x[m, kt*128 + p]; last M cols: ones row at p=0
    xsb = np.zeros((128, KT * M + M), dtype=np.float16)
    xsb[:, :KT * M] = x2.T.reshape(KT, 128, M).transpose(1, 0, 2).reshape(128, KT * M)
    xsb[0, KT * M:] = 1.0
    qweight = np.asarray(qweight)
    w_scales = np.asarray(w_scales, dtype=np.float16).reshape(N)
    bias = np.asarray(bias, dtype=np.float16).reshape(N)
    ucnt = _u_classes()
    in_maps = []
    for c in range(NCORES):
        sl = slice(c * NS, (c + 1) * NS)
        # A[kt, p, n] = q[n, kt*128+p]
        A = np.ascontiguousarray(qweight[sl, :].T).reshape(KT, 128, NS)
        qarr = {k: np.empty((n, 128, k[0] * k[1]), dtype=np.int8)
                for k, n in ucnt.items()}
        qidx = {k: 0 for k in ucnt}
        for c0, w, groups in SPANS:
            kt0 = 0
            for u in groups:
                blk = A[kt0:kt0 + u, :, c0:c0 + w].transpose(1, 0, 2)
                qarr[(u, w)][qidx[(u, w)]] = blk.reshape(128, u * w)
                qidx[(u, w)] += 1
                kt0 += u
        srep = np.broadcast_to(w_scales[sl], (M, NS)).astype(np.float16)
        # bias enters the PSUM accumulation before the scale multiply:
        # out = (sum x*q + b/s) * s
        bos = (bias[sl].astype(np.float32)
               / w_scales[sl].astype(np.float32)).astype(np.float16)
        im = {"x": xsb, "s": np.ascontiguousarray(srep),
              "b": np.ascontiguousarray(bos.reshape(1, NS))}
        for (u, w) in ucnt:
            im[f"q{u}w{w}"] = qarr[(u, w)]
        in_maps.append(im)
    return in_maps


def _run(x, qweight, w_scales, bias, trace=False):
    nc = _get_nc()
    in_maps = _prep_inputs(x, qweight, w_scales, bias)
    res = run_bass_kernel_spmd(
        nc, in_maps, core_ids=list(range(NCORES)), trace=trace
    )
    y = np.concatenate(
        [np.asarray(res.results[c]["out"]) for c in range(NCORES)], axis=1
    )
    return y.reshape(B, S, N).astype(np.float16), res


def kernel(x, qweight, w_scales, bias):
    y, _ = _run(x, qweight, w_scales, bias, trace=False)
    return y


def kernel_traced(x, qweight, w_scales, bias):
    """Like kernel() but also returns the BassKernelResults (exec_time_ns)."""
    return _run(x, qweight, w_scales, bias, trace=True)


# revision 34
# speedup vs baseline: 1.1448x; 1.0765x over previous
"""W8A16 column-parallel linear for TRN2, 8 NeuronCores.

Computes y = x @ (qweight * w_scales).T + bias with
  x        [8, 1, 8192]  fp16
  qweight  [28672, 8192] int8 (per-row symmetric quant)
  w_scales [28672, 1]    fp16
  bias     [28672]       fp16
  y        [8, 1, 28672] fp16

Sharding: column-parallel - each of the 8 cores owns 3584 output rows
(qweight/w_scales/bias shard), x replicated. No collectives; outputs are
concatenated on the host.

Per-core kernel, span-major streaming: the 3584 output columns are cut
into 7 spans of 512. The int8 weight shard streams span-by-span from HBM
(host pre-arranges each DMA group as a contiguous [128, u*512] slab so
descriptors are u*512 B per partition), is converted int8->fp16 on-chip
(k-split between VectorE 2x-port mode and ScalarE), and accumulates into
that span's PSUM bank with fp16 matmuls (stationary x^T tile, moving
weight tile). Each span is split into 2x256-col halves on different PE
column groups so even a HAM-cold PE outruns the DMA pace. When a span's
64 k-tiles finish, its scale-multiply (out=(sum x*q + b/s)*s) and output
DMA run *under* the next span's weight stream - unlike the k-major
baseline whose 3 serial full-width scale-muls + output DMAs formed an
~18 us tail after the last weight byte. The last span tapers its group
sizes so the drain after the final weight byte is short.

Measured on TRN2 (8 cores, neuron-profile total_time, median of 3):
~102.3us vs ~106us for the k-major baseline; rel err 3.4e-4; weight
stream gap-free at 99.4% of the 358 GB/s HBM line rate. Floor
decomposition: ~8.7us runtime startup + ~89-90us PE-paced stream +
~3us drain/out. The PE is the binding stage at ~2.58 weight-cols/ns
(~1.08 cols/cycle with the two-column-group pairing; the drain/fill +
LDW overlap is a pairwise-only effect worth ~75ns/pair).

Falsified on HW, do not retry blindly:
- mid-stream scale-muls or output DMAs (any op whose semaphore waits
  on the lagging PE stalls the strict-FIFO DVE queue / HWDGE
  sequencer, idles the PE >3.4us, the HAM clock gate halves its
  clock, and the pipeline death-spirals: +15-25us even when deferred
  by 2 spans; hence ALL muls/outs issue after the full stream)
- 3-way column-group splits at BOTH 171 cols (1.77 cols/ns) and
  512 cols / 1536-wide spans (110.1us; the pairwise overlap does not
  chain to a third stream)
- 1024-col moving tiles (s3d3_mm_num_elements ISA reject)
- --enable-ldw-opt=true (hard device hang once, slower when it ran)
- bf16 matmul path (identical 102.5us - no 2-elem/cycle moving path;
  fp16 kept for 5x better rel err)
- one merged/padded weight param (121us: slice-of-param APs degrade
  the DMA descriptor program; only whole-param rearrange views
  stream at line rate)
- standalone nc.tensor.ldweights + InstMatmult(ldweights=False)
  (the non-self-loading pattern builds and is CORRECT on HW, but
  measures 128.2us - the explicit LDW pairs serialize worse than the
  fused LDW+MM emission; keep fused matmul())
- deeper buffering wq=7/wf=6 (120.6us vs 102.3 at wq=6/wf=4: the
  ~20MB SBUF occupancy degrades engine access patterns; the pool
  depths are part of the tuned optimum, not just safety margin)

Remaining slack for a future session: the ~8.7us runtime startup
(below the NEFF/NRT layer reachable from bass) and the PE
moving-stream ceiling (needs xbus_sel control, not exposed in bass).
"""

import numpy as np

import concourse.bacc as bacc
import concourse.mybir as mybir
import concourse.tile as tile
from concourse.bass_utils import run_bass_kernel_spmd

B, S, K, N = 8, 1, 8192, 28672
M = B * S                 # 8 rows in the GEMM
NCORES = 8
NS = N // NCORES          # 3584 output rows per core
KT = K // 128             # 64 k-tiles
# spans 0-2 are 1024 cols (two 512-col halves on PE column groups 0/1 -
# alternating the two moving streams overlaps them ~50% on the PE);
# span 3 is 512 cols (two 256-col halves). Per-span k-group sizes: the
# last span tapers so the pipeline drains fast after the final byte.
SPANS = [(0, 1024, [8] * 8),
         (1024, 1024, [8] * 8),
         (2048, 1024, [8] * 8),
         (3072, 512, [16, 16, 16, 8, 4, 2, 1, 1])]
NSPAN = len(SPANS)
for _, _, gs in SPANS:
    assert sum(gs) == KT

# k-split of each group's int8->fp16 conversion: VectorE (2 elem/cyc)
# takes dve_u() k-tiles, ScalarE (1 elem/cyc @1.2GHz) the rest.
DVE_U = {(8, 1024): 5, (16, 512): 10, (8, 512): 5, (4, 512): 3,
         (2, 512): 2, (1, 512): 1}


def dve_u(u, w):
    return DVE_U[(u, w)]

_CACHE = {}


def _u_classes():
    """((u, width) -> number of groups of that class in the kernel)."""
    cnt = {}
    for _, w, gs in SPANS:
        for u in gs:
            cnt[(u, w)] = cnt.get((u, w), 0) + 1
    return cnt


def _build():
    # NOTE: do NOT flip --enable-ldw-opt=true: it hard-hangs the device
    # (NRT_EXEC_UNIT_UNRECOVERABLE) with this instruction pattern.
    nc = bacc.Bacc()
    xp = nc.declare_dram_parameter("x", [128, KT * M + M], mybir.dt.float16,
                                   isOutput=False)
    ucnt = _u_classes()
    qps = {
        (u, w): nc.declare_dram_parameter(f"q{u}w{w}", [n, 128, u * w],
                                          mybir.dt.int8, isOutput=False)
        for (u, w), n in sorted(ucnt.items())
    }
    sp_ = nc.declare_dram_parameter("s", [M, NS], mybir.dt.float16, isOutput=False)
    bp = nc.declare_dram_parameter("b", [1, NS], mybir.dt.float16, isOutput=False)
    op = nc.declare_dram_parameter("out", [M, NS], mybir.dt.float16, isOutput=True)
    # whole-param rearranges: keep the (u n) free dim contiguous per
    # partition so each group DMA is 128 descriptors of u*512 bytes
    qv = {k: qps[k].rearrange("g p (u n) -> g p u n", u=k[0]) for k in qps}

    with tile.TileContext(nc) as tc:
        with (
            tc.tile_pool(name="const", bufs=1) as constp,
            tc.tile_pool(name="wq", bufs=6) as wqp,
            tc.tile_pool(name="wf", bufs=4) as wfp,
            tc.tile_pool(name="psum", bufs=1, space="PSUM") as psp,
        ):
            xsb = constp.tile([128, KT * M + M], mybir.dt.float16, tag="xsb")
            sb = constp.tile([40, NS], mybir.dt.float16, tag="sb")
            b1 = constp.tile([1, NS], mybir.dt.float16, tag="b1")
            osb = constp.tile([40, NS], mybir.dt.float16, tag="osb")
            # ones row for the bias-opening matmuls lives in xsb's last
            # M columns (host packs 1.0 at partition 0 there)
            ones = xsb[0:1, KT * M:KT * M + M]

            psum = psp.tile([128, NS], mybir.dt.float32, tag="psum")

            # the weight stream is the binding resource: its first group
            # leads the HWDGE queue, constants ride behind it
            qidx = {k: 0 for k in ucnt}
            k0 = (SPANS[0][2][0], SPANS[0][1])
            wq0 = wqp.tile([128, k0[0], k0[1]], mybir.dt.int8, tag="wq")
            nc.sync.dma_start(wq0[:], qv[k0][0])
            qidx[k0] += 1
            nc.sync.dma_start(xsb[:], xp[:])
            nc.sync.dma_start(b1[:], bp[:])
            for j in range(2):
                nc.sync.dma_start(sb[32 * j:32 * j + M, :], sp_[:])

            def halves(sp):
                c, w, _ = SPANS[sp]
                return [(0, c, c + w // 2), (32, c + w // 2, c + w)]

            def span_muls(sp):
                for p, c0, c1 in halves(sp):
                    nc.vector.tensor_mul(
                        osb[p:p + M, c0:c1],
                        psum[p:p + M, c0:c1],
                        sb[p:p + M, c0:c1])

            for sp in range(NSPAN):
                c, w, groups = SPANS[sp]
                # bias rows open the accumulation: psum = ones^T @ (b/s)
                for p, c0, c1 in halves(sp):
                    nc.tensor.matmul(psum[p:p + M, c0:c1], ones,
                                     b1[:, c0:c1], start=True, stop=False)

                kt0 = 0
                for g, u in enumerate(groups):
                    if sp == 0 and g == 0:
                        wq = wq0
                    else:
                        wq = wqp.tile([128, u, w], mybir.dt.int8, tag="wq")
                        nc.sync.dma_start(wq[:], qv[(u, w)][qidx[(u, w)]])
                        qidx[(u, w)] += 1
                    wf = wfp.tile([128, u, w], mybir.dt.float16, tag="wf")
                    u1 = dve_u(u, w)
                    nc.vector.tensor_copy(wf[:, 0:u1, :], wq[:, 0:u1, :])
                    if u1 < u:
                        nc.scalar.activation(
                            wf[:, u1:u, :], wq[:, u1:u, :],
                            mybir.ActivationFunctionType.Copy,
                        )
                    for ui in range(u):
                        kt = kt0 + ui
                        last = kt == KT - 1
                        xt = xsb[:, kt * M:(kt + 1) * M]
                        # alternate the two halves' matmuls so the PE
                        # overlaps the two moving streams
                        for p, c0, c1 in halves(sp):
                            nc.tensor.matmul(
                                psum[p:p + M, c0:c1], xt,
                                wf[:, ui, c0 - c:c1 - c],
                                start=False, stop=last)
                    kt0 += u

            # ALL scale-muls run after the stream: the PE is the
            # slowest pipeline stage (~2.6 weight-cols/ns vs the DMA's
            # ~2.8), so a mid-stream mul on the strict-FIFO DVE queue
            # waits on the lagging PE and starves the conversions the
            # PE is waiting for (cold-clock death spiral, measured
            # +15us). At the end they stall nothing.
            for sp in range(NSPAN):
                span_muls(sp)

            # output DMAs are batched AFTER the whole stream: a dma_start
            # makes the issuing sequencer wait on its source semaphore, so
            # a per-span out DMA would stall HWDGE descriptor generation
            # for every weight group queued behind it (measured: ~25 us).
            # Here they stall nothing, and all but the last span's fire
            # immediately.
            for sp in range(NSPAN):
                for p, c0, c1 in halves(sp):
                    nc.sync.dma_start(op[:, c0:c1], osb[p:p + M, c0:c1])

    nc.compile()
    return nc


def _get_nc():
    if "nc" not in _CACHE:
        _CACHE["nc"] = _build()
    return _CACHE["nc"]


def _prep_inputs(x, qweight, w_scales, bias):
    x2 = np.asarray(x, dtype=np.float16).reshape(M, K)
    # xsb[p, kt*M + m] = x[m, kt*128 + p]; last M cols: ones row at p=0
    xsb = np.zeros((128, KT * M + M), dtype=np.float16)
    xsb[:, :KT * M] = x2.T.reshape(KT, 128, M).transpose(1, 0, 2).reshape(128, KT * M)
    xsb[0, KT * M:] = 1.0
    qweight = np.asarray(qweight)
    w_scales = np.asarray(w_scales, dtype=np.float16).reshape(N)
    bias = np.asarray(bias, dtype=np.float16).reshape(N)
    ucnt = _u_classes()
    in_maps = []
    for c in range(NCORES):
        sl = slice(c * NS, (c + 1) * NS)
        # A[kt, p, n] = q[n, kt*128+p]
        A = np.ascontiguousarray(qweight[sl, :].T).reshape(KT, 128, NS)
        qarr = {k: np.empty((n, 128, k[0] * k[1]), dtype=np.int8)
                for k, n in ucnt.items()}
        qidx = {k: 0 for k in ucnt}
        for c0, w, groups in SPANS:
            kt0 = 0
            for u in groups:
                blk = A[kt0:kt0 + u, :, c0:c0 + w].transpose(1, 0, 2)
                qarr[(u, w)][qidx[(u, w)]] = blk.reshape(128, u * w)
                qidx[(u, w)] += 1
                kt0 += u
        srep = np.broadcast_to(w_scales[sl], (M, NS)).astype(np.float16)
        # bias enters the PSUM accumulation before the scale multiply:
        # out = (sum x*q + b/s) * s
        bos = (bias[sl].astype(np.float32)
               / w_scales[sl].astype(np.float32)).astype(np.float16)
        im = {"x": xsb, "s": np.ascontiguousarray(srep),
              "b": np.ascontiguousarray(bos.reshape(1, NS))}
        for (u, w) in ucnt:
            im[f"q{u}w{w}"] = qarr[(u, w)]
        in_maps.append(im)
    return in_maps


def _run(x, qweight, w_scales, bias, trace=False):
    nc = _get_nc()
    in_maps = _prep_inputs(x, qweight, w_scales, bias)
    res = run_bass_kernel_spmd(
        nc, in_maps, core_ids=list(range(NCORES)), trace=trace
    )
    y = np.concatenate(
        [np.asarray(res.results[c]["out"]) for c in range(NCORES)], axis=1
    )
    return y.reshape(B, S, N).astype(np.float16), res


def kernel(x, qweight, w_scales, bias):
    y, _ = _run(x, qweight, w_scales, bias, trace=False)
    return y


def kernel_traced(x, qweight, w_scales, bias):
    """Like kernel() but also returns the BassKernelResults (exec_time_ns)."""
    return _run(x, qweight, w_scales, bias, trace=True)


# revision 36
# speedup vs baseline: 1.1926x; 1.0418x over previous
"""W8A16 column-parallel linear for TRN2, 8 NeuronCores.

Computes y = x @ (qweight * w_scales).T + bias with
  x        [8, 1, 8192]  fp16
  qweight  [28672, 8192] int8 (per-row symmetric quant)
  w_scales [28672, 1]    fp16
  bias     [28672]       fp16
  y        [8, 1, 28672] fp16

Sharding: column-parallel - each of the 8 cores owns 3584 output rows
(qweight/w_scales/bias shard), x replicated. No collectives; outputs are
concatenated on the host.

Per-core kernel, span-major streaming: the 3584 output columns are cut
into 7 spans of 512. The int8 weight shard streams span-by-span from HBM
(host pre-arranges each DMA group as a contiguous [128, u*512] slab so
descriptors are u*512 B per partition), is converted int8->fp16 on-chip
(k-split between VectorE 2x-port mode and ScalarE), and accumulates into
that span's PSUM bank with fp16 matmuls (stationary x^T tile, moving
weight tile). Each span is split into 2x256-col halves on different PE
column groups so even a HAM-cold PE outruns the DMA pace. When a span's
64 k-tiles finish, its scale-multiply (out=(sum x*q + b/s)*s) and output
DMA run *under* the next span's weight stream - unlike the k-major
baseline whose 3 serial full-width scale-muls + output DMAs formed an
~18 us tail after the last weight byte. The last span tapers its group
sizes so the drain after the final weight byte is short.

Measured on TRN2 (8 cores, neuron-profile total_time, median of 3):
~102.3us vs ~106us for the k-major baseline; rel err 3.4e-4 (best
sessions measure 102.1-102.6; the same binary has also measured
110-116 in noisy sessions - expect ~10% environmental run-to-run
drift from device-pool/HBM contention when comparing numbers); weight
stream gap-free at 99.4% of the 358 GB/s HBM line rate. Floor
decomposition: ~8.7us runtime startup + ~89-90us PE-paced stream +
~3us drain/out. The PE is the binding stage at ~2.58 weight-cols/ns
(~1.08 cols/cycle with the two-column-group pairing; the drain/fill +
LDW overlap is a pairwise-only effect worth ~75ns/pair).

Falsified on HW, do not retry blindly:
- mid-stream scale-muls or output DMAs (any op whose semaphore waits
  on the lagging PE stalls the strict-FIFO DVE queue / HWDGE
  sequencer, idles the PE >3.4us, the HAM clock gate halves its
  clock, and the pipeline death-spirals: +15-25us even when deferred
  by 2 spans; hence ALL muls/outs issue after the full stream)
- 3-way column-group splits at BOTH 171 cols (1.77 cols/ns) and
  512 cols / 1536-wide spans (110.1us; the pairwise overlap does not
  chain to a third stream)
- 1024-col moving tiles (s3d3_mm_num_elements ISA reject)
- --enable-ldw-opt=true (hard device hang once, slower when it ran)
- bf16 matmul path (identical 102.5us - no 2-elem/cycle moving path;
  fp16 kept for 5x better rel err)
- one merged/padded weight param (121us: slice-of-param APs degrade
  the DMA descriptor program; only whole-param rearrange views
  stream at line rate)
- standalone nc.tensor.ldweights + InstMatmult(ldweights=False)
  (the non-self-loading pattern builds and is CORRECT on HW, but
  measures 128.2us - the explicit LDW pairs serialize worse than the
  fused LDW+MM emission; keep fused matmul())
- deeper buffering wq=7/wf=6 (120.6us vs 102.3 at wq=6/wf=4: the
  ~20MB SBUF occupancy degrades engine access patterns; the pool
  depths are part of the tuned optimum, not just safety margin)

Remaining slack for a future session: the ~8.7us runtime startup
(below the NEFF/NRT layer reachable from bass) and the PE
moving-stream ceiling (needs xbus_sel control, not exposed in bass).
"""

import numpy as np

import concourse.bacc as bacc
import concourse.mybir as mybir
import concourse.tile as tile
from concourse.bass_utils import run_bass_kernel_spmd

B, S, K, N = 8, 1, 8192, 28672
M = B * S                 # 8 rows in the GEMM
NCORES = 8
NS = N // NCORES          # 3584 output rows per core
KT = K // 128             # 64 k-tiles
# spans 0-2 are 1024 cols (two 512-col halves on PE column groups 0/1 -
# alternating the two moving streams overlaps them ~50% on the PE);
# span 3 is 512 cols (two 256-col halves). Per-span k-group sizes: the
# last span tapers so the pipeline drains fast after the final byte.
SPANS = [(0, 1024, [8] * 8),
         (1024, 1024, [8] * 8),
         (2048, 1024, [8] * 8),
         (3072, 512, [16, 16, 16, 8, 4, 2, 1, 1])]
NSPAN = len(SPANS)
for _, _, gs in SPANS:
    assert sum(gs) == KT

# k-split of each group's int8->fp16 conversion: VectorE (2 elem/cyc)
# takes dve_u() k-tiles, ScalarE (1 elem/cyc @1.2GHz) the rest.
DVE_U = {(8, 1024): 5, (16, 512): 10, (8, 512): 5, (4, 512): 3,
         (2, 512): 2, (1, 512): 1}


def dve_u(u, w):
    return DVE_U[(u, w)]

_CACHE = {}


def _u_classes():
    """((u, width) -> number of groups of that class in the kernel)."""
    cnt = {}
    for _, w, gs in SPANS:
        for u in gs:
            cnt[(u, w)] = cnt.get((u, w), 0) + 1
    return cnt


def _build():
    # NOTE: do NOT flip --enable-ldw-opt=true: it hard-hangs the device
    # (NRT_EXEC_UNIT_UNRECOVERABLE) with this instruction pattern.
    nc = bacc.Bacc()
    xp = nc.declare_dram_parameter("x", [128, KT * M + M], mybir.dt.float16,
                                   isOutput=False)
    ucnt = _u_classes()
    qps = {
        (u, w): nc.declare_dram_parameter(f"q{u}w{w}", [n, 128, u * w],
                                          mybir.dt.int8, isOutput=False)
        for (u, w), n in sorted(ucnt.items())
    }
    sp_ = nc.declare_dram_parameter("s", [M, NS], mybir.dt.float16, isOutput=False)
    bp = nc.declare_dram_parameter("b", [1, NS], mybir.dt.float16, isOutput=False)
    op = nc.declare_dram_parameter("out", [M, NS], mybir.dt.float16, isOutput=True)
    # whole-param rearranges: keep the (u n) free dim contiguous per
    # partition so each group DMA is 128 descriptors of u*512 bytes
    qv = {k: qps[k].rearrange("g p (u n) -> g p u n", u=k[0]) for k in qps}

    with tile.TileContext(nc) as tc:
        with (
            tc.tile_pool(name="const", bufs=1) as constp,
            tc.tile_pool(name="wq", bufs=6) as wqp,
            tc.tile_pool(name="wf", bufs=4) as wfp,
            tc.tile_pool(name="psum", bufs=1, space="PSUM") as psp,
        ):
            xsb = constp.tile([128, KT * M + M], mybir.dt.float16, tag="xsb")
            sb = constp.tile([40, NS], mybir.dt.float16, tag="sb")
            b1 = constp.tile([1, NS], mybir.dt.float16, tag="b1")
            osb = constp.tile([40, NS], mybir.dt.float16, tag="osb")
            # ones row for the bias-opening matmuls lives in xsb's last
            # M columns (host packs 1.0 at partition 0 there)
            ones = xsb[0:1, KT * M:KT * M + M]

            # 8th PSUM bank (cols 3584:4096) holds only PE-warmup junk
            psum = psp.tile([128, NS + 512], mybir.dt.float32, tag="psum")

            # b1 (7KB, one descriptor) leads the queue so the PE warmup
            # below has data ~3.4us before the first converted weights;
            # the weight stream starts ~0.1us later for a ~1.7us win
            nc.sync.dma_start(b1[:], bp[:])
            qidx = {k: 0 for k in ucnt}
            k0 = (SPANS[0][2][0], SPANS[0][1])
            wq0 = wqp.tile([128, k0[0], k0[1]], mybir.dt.int8, tag="wq")
            nc.sync.dma_start(wq0[:], qv[k0][0])
            qidx[k0] += 1
            nc.sync.dma_start(xsb[:], xp[:])
            for j in range(2):
                nc.sync.dma_start(sb[32 * j:32 * j + M, :], sp_[:])

            # PE warmup: ~3.4us of junk matmuls into the spare PSUM bank
            # while the first weight group streams/converts, so the HAM
            # clock gate reaches 8/8 before the first real matmul instead
            # of halving the PE clock for its first ~3.4us (~1.7us saved)
            for _ in range(16):
                nc.tensor.matmul(psum[0:M, NS:NS + 512],
                                 b1[0:1, 0:M], b1[0:1, 0:512],
                                 start=True, stop=True)

            def halves(sp):
                c, w, _ = SPANS[sp]
                return [(0, c, c + w // 2), (32, c + w // 2, c + w)]

            def span_muls(sp):
                for p, c0, c1 in halves(sp):
                    nc.vector.tensor_mul(
                        osb[p:p + M, c0:c1],
                        psum[p:p + M, c0:c1],
                        sb[p:p + M, c0:c1])

            for sp in range(NSPAN):
                c, w, groups = SPANS[sp]
                # bias rows open the accumulation: psum = ones^T @ (b/s)
                for p, c0, c1 in halves(sp):
                    nc.tensor.matmul(psum[p:p + M, c0:c1], ones,
                                     b1[:, c0:c1], start=True, stop=False)

                kt0 = 0
                for g, u in enumerate(groups):
                    if sp == 0 and g == 0:
                        wq = wq0
                    else:
                        wq = wqp.tile([128, u, w], mybir.dt.int8, tag="wq")
                        nc.sync.dma_start(wq[:], qv[(u, w)][qidx[(u, w)]])
                        qidx[(u, w)] += 1
                    wf = wfp.tile([128, u, w], mybir.dt.float16, tag="wf")
                    u1 = dve_u(u, w)
                    nc.vector.tensor_copy(wf[:, 0:u1, :], wq[:, 0:u1, :])
                    if u1 < u:
                        nc.scalar.activation(
                            wf[:, u1:u, :], wq[:, u1:u, :],
                            mybir.ActivationFunctionType.Copy,
                        )
                    for ui in range(u):
                        kt = kt0 + ui
                        last = kt == KT - 1
                        xt = xsb[:, kt * M:(kt + 1) * M]
                        # alternate the two halves' matmuls so the PE
                        # overlaps the two moving streams
                        for p, c0, c1 in halves(sp):
                            nc.tensor.matmul(
                                psum[p:p + M, c0:c1], xt,
                                wf[:, ui, c0 - c:c1 - c],
                                start=False, stop=last)
                    kt0 += u

            # ALL scale-muls run after the stream: the PE is the
            # slowest pipeline stage (~2.6 weight-cols/ns vs the DMA's
            # ~2.8), so a mid-stream mul on the strict-FIFO DVE queue
            # waits on the lagging PE and starves the conversions the
            # PE is waiting for (cold-clock death spiral, measured
            # +15us). At the end they stall nothing.
            for sp in range(NSPAN):
                span_muls(sp)

            # output DMAs are batched AFTER the whole stream: a dma_start
            # makes the issuing sequencer wait on its source semaphore, so
            # a per-span out DMA would stall HWDGE descriptor generation
            # for every weight group queued behind it (measured: ~25 us).
            # Here they stall nothing, and all but the last span's fire
            # immediately.
            for sp in range(NSPAN):
                for p, c0, c1 in halves(sp):
                    nc.sync.dma_start(op[:, c0:c1], osb[p:p + M, c0:c1])

    nc.compile()
    return nc


def _get_nc():
    if "nc" not in _CACHE:
        _CACHE["nc"] = _build()
    return _CACHE["nc"]


def _prep_inputs(x, qweight, w_scales, bias):
    x2 = np.asarray(x, dtype=np.float16).reshape(M, K)
    # xsb[p, kt*M + m] = x[m, kt*128 + p]; last M cols: ones row at p=0
    xsb = np.zeros((128, KT * M + M), dtype=np.float16)
    xsb[:, :KT * M] = x2.T.reshape(KT, 128, M).transpose(1, 0, 2).reshape(128, KT * M)
    xsb[0, KT * M:] = 1.0
    qweight = np.asarray(qweight)
    w_scales = np.asarray(w_scales, dtype=np.float16).reshape(N)
    bias = np.asarray(bias, dtype=np.float16).reshape(N)
    ucnt = _u_classes()
    in_maps = []
    for c in range(NCORES):
        sl = slice(c * NS, (c + 1) * NS)
        # A[kt, p, n] = q[n, kt*128+p]
        A = np.ascontiguousarray(qweight[sl, :].T).reshape(KT, 128, NS)
        qarr = {k: np.empty((n, 128, k[0] * k[1]), dtype=np.int8)
                for k, n in ucnt.items()}
        qidx = {k: 0 for k in ucnt}
        for c0, w, groups in SPANS:
            kt0 = 0
            for u in groups:
                blk = A[kt0:kt0 + u, :, c0:c0 + w].transpose(1, 0, 2)
                qarr[(u, w)][qidx[(u, w)]] = blk.reshape(128, u * w)
                qidx[(u, w)] += 1
                kt0 += u
        srep = np.broadcast_to(w_scales[sl], (M, NS)).astype(np.float16)
        # bias enters the PSUM accumulation before the scale multiply:
        # out = (sum x*q + b/s) * s
        bos = (bias[sl].astype(np.float32)
               / w_scales[sl].astype(np.float32)).astype(np.float16)
        im = {"x": xsb, "s": np.ascontiguousarray(srep),
              "b": np.ascontiguousarray(bos.reshape(1, NS))}
        for (u, w) in ucnt:
            im[f"q{u}w{w}"] = qarr[(u, w)]
        in_maps.append(im)
    return in_maps


def _run(x, qweight, w_scales, bias, trace=False):
    nc = _get_nc()
    in_maps = _prep_inputs(x, qweight, w_scales, bias)
    res = run_bass_kernel_spmd(
        nc, in_maps, core_ids=list(range(NCORES)), trace=trace
    )
    y = np.concatenate(
        [np.asarray(res.results[c]["out"]) for c in range(NCORES)], axis=1
    )
    return y.reshape(B, S, N).astype(np.float16), res


def kernel(x, qweight, w_scales, bias):
    y, _ = _run(x, qweight, w_scales, bias, trace=False)
    return y


def kernel_traced(x, qweight, w_scales, bias):
    """Like kernel() but also returns the BassKernelResults (exec_time_ns)."""
    return _run(x, qweight, w_scales, bias, trace=True)


# revision 37
# speedup vs baseline: 1.2143x; 1.0182x over previous
"""W8A16 column-parallel linear for TRN2, 8 NeuronCores.

Computes y = x @ (qweight * w_scales).T + bias with
  x        [8, 1, 8192]  fp16
  qweight  [28672, 8192] int8 (per-row symmetric quant)
  w_scales [28672, 1]    fp16
  bias     [28672]       fp16
  y        [8, 1, 28672] fp16

Sharding: column-parallel - each of the 8 cores owns 3584 output rows
(qweight/w_scales/bias shard), x replicated. No collectives; outputs are
concatenated on the host.

Per-core kernel, span-major streaming: the 3584 output columns are cut
into 7 spans of 512. The int8 weight shard streams span-by-span from HBM
(host pre-arranges each DMA group as a contiguous [128, u*512] slab so
descriptors are u*512 B per partition), is converted int8->fp16 on-chip
(k-split between VectorE 2x-port mode and ScalarE), and accumulates into
that span's PSUM bank with fp16 matmuls (stationary x^T tile, moving
weight tile). Each span is split into 2x256-col halves on different PE
column groups so even a HAM-cold PE outruns the DMA pace. When a span's
64 k-tiles finish, its scale-multiply (out=(sum x*q + b/s)*s) and output
DMA run *under* the next span's weight stream - unlike the k-major
baseline whose 3 serial full-width scale-muls + output DMAs formed an
~18 us tail after the last weight byte. The last span tapers its group
sizes so the drain after the final weight byte is short.

Measured on TRN2 (8 cores, neuron-profile total_time, median of 3):
~102.3us pre-warmup / ~100.6us expected with the PE pre-warm, vs
~106us for the k-major baseline; rel err 3.4e-4. Expect ~10%
environmental session-to-session drift (the same binary measured
102.1-102.6 in clean sessions and 110-116 in noisy ones); the PE
pre-warm A/B in one contemporaneous noisy window measured 103.3/
107.5/109.6 vs 110.0/112.0/116.0 without it. Weight
stream gap-free at 99.4% of the 358 GB/s HBM line rate. Floor
decomposition: ~8.7us runtime startup + ~89-90us PE-paced stream +
~3us drain/out. The PE is the binding stage at ~2.58 weight-cols/ns
(~1.08 cols/cycle with the two-column-group pairing; the drain/fill +
LDW overlap is a pairwise-only effect worth ~75ns/pair).

Falsified on HW, do not retry blindly:
- mid-stream scale-muls or output DMAs (any op whose semaphore waits
  on the lagging PE stalls the strict-FIFO DVE queue / HWDGE
  sequencer, idles the PE >3.4us, the HAM clock gate halves its
  clock, and the pipeline death-spirals: +15-25us even when deferred
  by 2 spans; hence ALL muls/outs issue after the full stream)
- 3-way column-group splits at BOTH 171 cols (1.77 cols/ns) and
  512 cols / 1536-wide spans (110.1us; the pairwise overlap does not
  chain to a third stream)
- 1024-col moving tiles (s3d3_mm_num_elements ISA reject)
- --enable-ldw-opt=true (hard device hang once, slower when it ran)
- bf16 matmul path (identical 102.5us - no 2-elem/cycle moving path;
  fp16 kept for 5x better rel err)
- one merged/padded weight param (121us: slice-of-param APs degrade
  the DMA descriptor program; only whole-param rearrange views
  stream at line rate)
- standalone nc.tensor.ldweights + InstMatmult(ldweights=False)
  (the non-self-loading pattern builds and is CORRECT on HW, but
  measures 128.2us - the explicit LDW pairs serialize worse than the
  fused LDW+MM emission; keep fused matmul())
- deeper buffering wq=7/wf=6 (120.6us vs 102.3 at wq=6/wf=4: the
  ~20MB SBUF occupancy degrades engine access patterns; the pool
  depths are part of the tuned optimum, not just safety margin)

The PE pre-warm: the first real matmul lands ~10.4us in (startup +
first group's DMA + conversion), and the HAM clock gate holds a
cold PE at 1.2 GHz for its first ~3.4us of activity. 16 junk K=1
matmuls into the spare 8th PSUM bank, fed by the bias row DMA'd
first, occupy exactly the pre-stream idle window so the real
stream starts at 2.4 GHz (~1.7us theoretical).

Remaining slack for a future session: the ~8.7us runtime startup
(below the NEFF/NRT layer reachable from bass) and the PE
moving-stream ceiling (needs xbus_sel control, not exposed in bass).
"""

import numpy as np

import concourse.bacc as bacc
import concourse.mybir as mybir
import concourse.tile as tile
from concourse.bass_utils import run_bass_kernel_spmd

B, S, K, N = 8, 1, 8192, 28672
M = B * S                 # 8 rows in the GEMM
NCORES = 8
NS = N // NCORES          # 3584 output rows per core
KT = K // 128             # 64 k-tiles
# spans 0-2 are 1024 cols (two 512-col halves on PE column groups 0/1 -
# alternating the two moving streams overlaps them ~50% on the PE);
# span 3 is 512 cols (two 256-col halves). Per-span k-group sizes: the
# last span tapers so the pipeline drains fast after the final byte.
SPANS = [(0, 1024, [8] * 8),
         (1024, 1024, [8] * 8),
         (2048, 1024, [8] * 8),
         (3072, 512, [16, 16, 16, 8, 4, 2, 1, 1])]
NSPAN = len(SPANS)
for _, _, gs in SPANS:
    assert sum(gs) == KT

# k-split of each group's int8->fp16 conversion: VectorE (2 elem/cyc)
# takes dve_u() k-tiles, ScalarE (1 elem/cyc @1.2GHz) the rest.
DVE_U = {(8, 1024): 5, (16, 512): 10, (8, 512): 5, (4, 512): 3,
         (2, 512): 2, (1, 512): 1}


def dve_u(u, w):
    return DVE_U[(u, w)]

_CACHE = {}


def _u_classes():
    """((u, width) -> number of groups of that class in the kernel)."""
    cnt = {}
    for _, w, gs in SPANS:
        for u in gs:
            cnt[(u, w)] = cnt.get((u, w), 0) + 1
    return cnt


def _build():
    # NOTE: do NOT flip --enable-ldw-opt=true: it hard-hangs the device
    # (NRT_EXEC_UNIT_UNRECOVERABLE) with this instruction pattern.
    nc = bacc.Bacc()
    xp = nc.declare_dram_parameter("x", [128, KT * M + M], mybir.dt.float16,
                                   isOutput=False)
    ucnt = _u_classes()
    qps = {
        (u, w): nc.declare_dram_parameter(f"q{u}w{w}", [n, 128, u * w],
                                          mybir.dt.int8, isOutput=False)
        for (u, w), n in sorted(ucnt.items())
    }
    sp_ = nc.declare_dram_parameter("s", [M, NS], mybir.dt.float16, isOutput=False)
    bp = nc.declare_dram_parameter("b", [1, NS], mybir.dt.float16, isOutput=False)
    op = nc.declare_dram_parameter("out", [M, NS], mybir.dt.float16, isOutput=True)
    # whole-param rearranges: keep the (u n) free dim contiguous per
    # partition so each group DMA is 128 descriptors of u*512 bytes
    qv = {k: qps[k].rearrange("g p (u n) -> g p u n", u=k[0]) for k in qps}

    with tile.TileContext(nc) as tc:
        with (
            tc.tile_pool(name="const", bufs=1) as constp,
            tc.tile_pool(name="wq", bufs=6) as wqp,
            tc.tile_pool(name="wf", bufs=4) as wfp,
            tc.tile_pool(name="psum", bufs=1, space="PSUM") as psp,
        ):
            xsb = constp.tile([128, KT * M + M], mybir.dt.float16, tag="xsb")
            sb = constp.tile([40, NS], mybir.dt.float16, tag="sb")
            b1 = constp.tile([1, NS], mybir.dt.float16, tag="b1")
            osb = constp.tile([40, NS], mybir.dt.float16, tag="osb")
            # ones row for the bias-opening matmuls lives in xsb's last
            # M columns (host packs 1.0 at partition 0 there)
            ones = xsb[0:1, KT * M:KT * M + M]

            # 8th PSUM bank (cols 3584:4096) holds only PE-warmup junk
            psum = psp.tile([128, NS + 512], mybir.dt.float32, tag="psum")

            # b1 (7KB, one descriptor) leads the queue so the PE warmup
            # below has data ~3.4us before the first converted weights;
            # the weight stream starts ~0.1us later for a ~1.7us win
            nc.sync.dma_start(b1[:], bp[:])
            qidx = {k: 0 for k in ucnt}
            k0 = (SPANS[0][2][0], SPANS[0][1])
            wq0 = wqp.tile([128, k0[0], k0[1]], mybir.dt.int8, tag="wq")
            nc.sync.dma_start(wq0[:], qv[k0][0])
            qidx[k0] += 1
            nc.sync.dma_start(xsb[:], xp[:])
            for j in range(2):
                nc.sync.dma_start(sb[32 * j:32 * j + M, :], sp_[:])

            # PE warmup: ~3.4us of junk matmuls into the spare PSUM bank
            # while the first weight group streams/converts, so the HAM
            # clock gate reaches 8/8 before the first real matmul instead
            # of halving the PE clock for its first ~3.4us (~1.7us saved)
            for _ in range(16):
                nc.tensor.matmul(psum[0:M, NS:NS + 512],
                                 b1[0:1, 0:M], b1[0:1, 0:512],
                                 start=True, stop=True)

            def halves(sp):
                c, w, _ = SPANS[sp]
                return [(0, c, c + w // 2), (32, c + w // 2, c + w)]

            def span_muls(sp):
                for p, c0, c1 in halves(sp):
                    nc.vector.tensor_mul(
                        osb[p:p + M, c0:c1],
                        psum[p:p + M, c0:c1],
                        sb[p:p + M, c0:c1])

            for sp in range(NSPAN):
                c, w, groups = SPANS[sp]
                # bias rows open the accumulation: psum = ones^T @ (b/s)
                for p, c0, c1 in halves(sp):
                    nc.tensor.matmul(psum[p:p + M, c0:c1], ones,
                                     b1[:, c0:c1], start=True, stop=False)

                kt0 = 0
                for g, u in enumerate(groups):
                    if sp == 0 and g == 0:
                        wq = wq0
                    else:
                        wq = wqp.tile([128, u, w], mybir.dt.int8, tag="wq")
                        nc.sync.dma_start(wq[:], qv[(u, w)][qidx[(u, w)]])
                        qidx[(u, w)] += 1
                    wf = wfp.tile([128, u, w], mybir.dt.float16, tag="wf")
                    u1 = dve_u(u, w)
                    nc.vector.tensor_copy(wf[:, 0:u1, :], wq[:, 0:u1, :])
                    if u1 < u:
                        nc.scalar.activation(
                            wf[:, u1:u, :], wq[:, u1:u, :],
                            mybir.ActivationFunctionType.Copy,
                        )
                    for ui in range(u):
                        kt = kt0 + ui
                        last = kt == KT - 1
                        xt = xsb[:, kt * M:(kt + 1) * M]
                        # alternate the two halves' matmuls so the PE
                        # overlaps the two moving streams
                        for p, c0, c1 in halves(sp):
                            nc.tensor.matmul(
                                psum[p:p + M, c0:c1], xt,
                                wf[:, ui, c0 - c:c1 - c],
                                start=False, stop=last)
                    kt0 += u

            # ALL scale-muls run after the stream: the PE is the
            # slowest pipeline stage (~2.6 weight-cols/ns vs the DMA's
            # ~2.8), so a mid-stream mul on the strict-FIFO DVE queue
            # waits on the lagging PE and starves the conversions the
            # PE is waiting for (cold-clock death spiral, measured
            # +15us). At the end they stall nothing.
            for sp in range(NSPAN):
                span_muls(sp)

            # output DMAs are batched AFTER the whole stream: a dma_start
            # makes the issuing sequencer wait on its source semaphore, so
            # a per-span out DMA would stall HWDGE descriptor generation
            # for every weight group queued behind it (measured: ~25 us).
            # Here they stall nothing, and all but the last span's fire
            # immediately.
            for sp in range(NSPAN):
                for p, c0, c1 in halves(sp):
                    nc.sync.dma_start(op[:, c0:c1], osb[p:p + M, c0:c1])

    nc.compile()
    return nc


def _get_nc():
    if "nc" not in _CACHE:
        _CACHE["nc"] = _build()
    return _CACHE["nc"]


def _prep_inputs(x, qweight, w_scales, bias):
    x2 = np.asarray(x, dtype=np.float16).reshape(M, K)
    # xsb[p, kt*M + m] = x[m, kt*128 + p]; last M cols: ones row at p=0
    xsb = np.zeros((128, KT * M + M), dtype=np.float16)
    xsb[:, :KT * M] = x2.T.reshape(KT, 128, M).transpose(1, 0, 2).reshape(128, KT * M)
    xsb[0, KT * M:] = 1.0
    qweight = np.asarray(qweight)
    w_scales = np.asarray(w_scales, dtype=np.float16).reshape(N)
    bias = np.asarray(bias, dtype=np.float16).reshape(N)
    ucnt = _u_classes()
    in_maps = []
    for c in range(NCORES):
        sl = slice(c * NS, (c + 1) * NS)
        # A[kt, p, n] = q[n, kt*128+p]
        A = np.ascontiguousarray(qweight[sl, :].T).reshape(KT, 128, NS)
        qarr = {k: np.empty((n, 128, k[0] * k[1]), dtype=np.int8)
                for k, n in ucnt.items()}
        qidx = {k: 0 for k in ucnt}
        for c0, w, groups in SPANS:
            kt0 = 0
            for u in groups:
                blk = A[kt0:kt0 + u, :, c0:c0 + w].transpose(1, 0, 2)
                qarr[(u, w)][qidx[(u, w)]] = blk.reshape(128, u * w)
                qidx[(u, w)] += 1
                kt0 += u
        srep = np.broadcast_to(w_scales[sl], (M, NS)).astype(np.float16)
        # bias enters the PSUM accumulation before the scale multiply:
        # out = (sum x*q + b/s) * s
        bos = (bias[sl].astype(np.float32)
               / w_scales[sl].astype(np.float32)).astype(np.float16)
        im = {"x": xsb, "s": np.ascontiguousarray(srep),
              "b": np.ascontiguousarray(bos.reshape(1, NS))}
        for (u, w) in ucnt:
            im[f"q{u}w{w}"] = qarr[(u, w)]
        in_maps.append(im)
    return in_maps


def _run(x, qweight, w_scales, bias, trace=False):
    nc = _get_nc()
    in_maps = _prep_inputs(x, qweight, w_scales, bias)
    res = run_bass_kernel_spmd(
        nc, in_maps, core_ids=list(range(NCORES)), trace=trace
    )
    y = np.concatenate(
        [np.asarray(res.results[c]["out"]) for c in range(NCORES)], axis=1
    )
    return y.reshape(B, S, N).astype(np.float16), res


def kernel(x, qweight, w_scales, bias):
    y, _ = _run(x, qweight, w_scales, bias, trace=False)
    return y


def kernel_traced(x, qweight, w_scales, bias):
    """Like kernel() but also returns the BassKernelResults (exec_time_ns)."""
    return _run(x, qweight, w_scales, bias, trace=True)
